# revision 57
# baseline (speedup 1.0000x reference)
"""Self-contained Trainium2 Bass kernel for nn_DisGNN (CGConv GNN), 8-core SPMD.

v16 (~2.13 ms vs 2.92 ms baseline). Phase-A DMAs batched over 4-window
groups (HWDGE charges ~630ns fixed per dma_start; startup was
HWDGE-bound). Edge-parallel by dst core; per conv the
gate pre-activations accumulate in PSUM: edge-MLP matmul + fp8 identity-inject
of DMA-gathered source tables, with the dst-table add and all remaining
elementwise work on DVE restricted to fast-mode TT/TS ops (STT gets no 2x/4x
mode). Node tables are fp8e3 (halves gather traffic + AllGather payload);
conv2's gate weights are host-scaled x0.25 to stay in fp8 range, undone in the
activation scales. Scatter-add is a one-hot matmul per 128-edge block with
block counts set per-window (max over cores, ~9% less padding than a global
max). One-hot matrices build once at startup (hidden under AllGather #1),
stashed in DRAM as fp8e4 -- mixed fp8e4-lhsT x bf16-rhs matmul works on this
hw, halving the stash traffic; edge attrs ship as fp8e3. The
blob loopback copy issues after phase A so it never blocks constant loads.

Measured notes for this axon backend: chained timing has a ~0.65 ms/exec
dispatch floor; every extra ExternalOutput costs ~1.4 ms/exec (tunnel
round-trip); collectives are ~50 us each (cost model says 284); real
per-matmul issue overhead is several times the model, so PSUM-inject designs
that triple matmul count lose despite modeled DVE savings; SWDGE gather calls
hard-cap at 1024 descriptors regardless of dynamic_dma_scratch_size.
"""
import sys, os
for p in ('/opt/trn_rl_repo', '/root/.axon_site/_ro/trn_rl_repo'):
    if os.path.isdir(p) and p not in sys.path:
        sys.path.insert(0, p)
import contextlib
import numpy as np
import ml_dtypes

# ======================= constants =======================

N, E, C, D, NCLS, G = 50000, 800000, 128, 32, 10, 64
NCORE = 8
NPC = 6272              # nodes per core (49*128)
WPC = 49                # windows per core
NPAD = NCORE * NPC      # 50176
S0 = 32640              # src table split (255*128) to keep int16 indices
CH = 1024               # gather call chunk (slots)

FP8NP = ml_dtypes.float8_e3m4
BF16NP = ml_dtypes.bfloat16


def blob_layout(SL, NBLK, Bw):
    """(offset, partitions, cols, elem_size) for every packed tensor."""
    L = {}
    off = 0
    def add(name, p, c, esz):
        nonlocal off
        off = (off + 511) // 512 * 512
        L[name] = (off, p, c, esz)
        off += p * c * esz
    add('eaT', WPC * 33, Bw * 128, 1)    # fp8e3 edge attrs (+ones row), slot order
    add('xT', C, NPC, 2)                 # bf16 node features (transposed)
    add('clsOH', 11, NPC, 2)             # bf16 one-hot class (row 0 = padding)
    add('srcs', 16, SL // 16, 2)         # i16 gather idx slab (wrapped cols)
    add('dsts', 16, SL // 16, 2)
    add('dstloc', 128, NBLK, 2)          # bf16 dst%128 per slot (255 = pad)
    add('deg', 128, WPC, 2)              # bf16 in-degree per own node
    add('bt', 128, WPC, 2)               # bf16 graph id per own node (255 = pad)
    add('invcnt', 128, G, 4)             # f32 1/count per graph (row-replicated)
    add('W1', C, C, 2)
    add('B10', 11, C, 2)
    add('Wtab1', C, 512, 2)
    add('Wtab2', C, 512, 2)
    add('Wea1', 33, 256, 2)
    add('Wea2', 33, 256, 2)
    add('fc1a', C, 32, 4)
    add('fc1b', NCLS, 32, 4)
    add('fc1bias', 1, 32, 4)
    add('fc2aug', 33, 1, 4)
    add('onehotT', NCLS, G, 4)
    add('ones64', 1, G, 4)
    add('res', 1, G, 4)       # result region (device-written, not shipped)
    total = (off + 511) // 512 * 512
    return L, total


# ======================= host preprocessing =======================

def prep(inputs):
    x = np.asarray(inputs['x'], np.float32)
    y = np.asarray(inputs['y']).astype(np.int64)
    edge_index = np.asarray(inputs['edge_index']).astype(np.int64)
    ea = np.asarray(inputs['edge_attr'], np.float32)
    batch = np.asarray(inputs['batch']).astype(np.int64)

    src, dst = edge_index[0], edge_index[1]
    cls = y[batch]
    cnt = np.bincount(batch, minlength=G).astype(np.float32)

    # ---- per (core, window, src-half) edge buckets ----
    core_of = dst // NPC
    win_of = (dst % NPC) // 128
    keys = (core_of * WPC + win_of) * 2 + (src >= S0).astype(np.int64)
    orderd = np.argsort(keys, kind='stable')
    ks = keys[orderd]
    bounds = np.searchsorted(ks, np.arange(NCORE * WPC * 2 + 1))
    # per-window block counts (max over cores only; the SPMD program is
    # shared across cores, so per-window bounds must agree core-to-core)
    csz = (bounds[1:] - bounds[:-1]).reshape(NCORE, WPC, 2)
    lowW = ((csz[:, :, 0] + 127) // 128).max(axis=0).astype(int)
    highW = ((csz[:, :, 1] + 127) // 128).max(axis=0).astype(int)
    lowB, highB = int(lowW.max()), int(highW.max())
    Bw = lowB + highB
    SPW = Bw * 128
    NBLK = WPC * Bw
    SL = NBLK * 128

    srcidx = np.zeros((NCORE, SL), np.int16)
    dstidx = np.zeros((NCORE, SL), np.int16)
    dstloc = np.full((NCORE, SL), 255.0, np.float32)
    easlot = np.zeros((NCORE, SL, D), np.float32)
    for c in range(NCORE):
        for w in range(WPC):
            k = (c * WPC + w) * 2
            elo = orderd[bounds[k]:bounds[k + 1]]
            ehi = orderd[bounds[k + 1]:bounds[k + 2]]
            base = w * SPW
            srcidx[c, base:base + len(elo)] = src[elo]
            dstidx[c, base:base + len(elo)] = dst[elo] - c * NPC
            dstloc[c, base:base + len(elo)] = dst[elo] % 128
            easlot[c, base:base + len(elo)] = ea[elo]
            hbase = base + int(lowW[w]) * 128
            srcidx[c, hbase:hbase + len(ehi)] = src[ehi] - S0
            dstidx[c, hbase:hbase + len(ehi)] = dst[ehi] - c * NPC
            dstloc[c, hbase:hbase + len(ehi)] = dst[ehi] % 128
            easlot[c, hbase:hbase + len(ehi)] = ea[ehi]

    # ---- gather idx slabs [16, SL/16] + call schedules (uniform across cores)
    def wrap16(iv):
        return iv.reshape(-1, 16).T.copy()

    def build_call_slab16(iv, chunk):
        cols, calls, off = [], [], 0
        for s in range(0, len(iv), chunk):
            piece = iv[s:s + chunk]
            cols.append(wrap16(piece))
            calls.append((off, len(piece)))
            off += len(piece) // 16
        return np.concatenate(cols, axis=1), calls

    src_slabs, dst_slabs = [], []
    src_calls, dst_calls = None, None
    wbase16, dbase16 = [], []
    for c in range(NCORE):
        scols, dcols = [], []
        src_calls, dst_calls = [], []
        wbase16, dbase16 = [], []
        for w in range(WPC):
            base = w * SPW
            lw, hw_ = int(lowW[w]), int(highW[w])
            wbase16.append(sum(s.shape[1] for s in scols))
            lo = srcidx[c, base:base + lw * 128]
            hi = srcidx[c, base + lw * 128:base + (lw + hw_) * 128]
            sl_, cl = build_call_slab16(lo, CH)
            off0 = sum(s.shape[1] for s in scols)
            scols.append(sl_)
            wcalls = [(off0 + o, n, 0) for (o, n) in cl]
            sl_, cl = build_call_slab16(hi, CH)
            off0 = sum(s.shape[1] for s in scols)
            scols.append(sl_)
            wcalls += [(off0 + o, n, 1) for (o, n) in cl]
            src_calls.append(wcalls)
            dbase16.append(sum(d.shape[1] for d in dcols))
            dl, cl = build_call_slab16(dstidx[c, base:base + (lw + hw_) * 128],
                                       CH)
            off0 = sum(d.shape[1] for d in dcols)
            dcols.append(dl)
            dst_calls.append([(off0 + o, n) for (o, n) in cl])
        spad = SL // 16 - sum(s.shape[1] for s in scols)
        scols.append(np.zeros((16, spad), np.int16))
        dpad = SL // 16 - sum(d.shape[1] for d in dcols)
        dcols.append(np.zeros((16, dpad), np.int16))
        src_slabs.append(np.concatenate(scols, axis=1))
        dst_slabs.append(np.concatenate(dcols, axis=1))
    src_slab = np.stack(src_slabs)      # [NCORE, 16, SL/16]
    dst_slab = np.stack(dst_slabs)

    # dstloc arranged [128, NBLK]: slot i -> [i%128, i//128]
    dstloc_a = dstloc.reshape(NCORE, NBLK, 128).transpose(0, 2, 1)
    # eaT [WPC*33, Bw*128]: row w*33+p, col b*128+j = easlot[c, (w*Bw+b)*128+j, p]
    eaT = np.ones((NCORE, WPC, 33, Bw * 128), np.float32)
    eaT[:, :, :32, :] = easlot.reshape(NCORE, WPC, Bw, 128, D).transpose(
        0, 1, 4, 2, 3).reshape(NCORE, WPC, D, Bw * 128)

    # node features / classes
    xT_full = np.zeros((C, NPAD), np.float32)
    xT_full[:, :N] = x.T
    clsidx = np.zeros(NPAD, np.int64)
    clsidx[:N] = cls + 1
    clsOH = np.zeros((11, NPAD), np.float32)
    clsOH[clsidx, np.arange(NPAD)] = 1.0

    indeg = np.bincount(dst, minlength=N).astype(np.float32)
    degp = np.zeros(NPAD, np.float32)
    degp[:N] = indeg
    gl = np.full(NPAD, 255.0, np.float32)
    gl[:N] = batch

    invcnt = (1.0 / np.maximum(cnt, 1.0)).astype(np.float32)
    invcnt128 = np.tile(invcnt[None, :], (128, 1)).astype(np.float32)

    W = {k: np.asarray(inputs[k], np.float32) for k in
         ['lin_W', 'lin_b', 'c1_Wf', 'c1_bf', 'c1_Ws', 'c1_bs', 'c2_Wf',
          'c2_bf', 'c2_Ws', 'c2_bs', 'fc1_W', 'fc1_b', 'fc2_W', 'fc2_b']}
    B10 = np.zeros((11, C), np.float32)
    B10[1:] = W['lin_W'][C:] + W['lin_b']

    def wtab(k):
        Wf, Ws = W[f'c{k}_Wf'], W[f'c{k}_Ws']
        return np.concatenate([Wf[:C], Ws[:C], Wf[C:2 * C], Ws[C:2 * C]], axis=1)

    def wea(k):
        Wf, Ws, bf, bs = W[f'c{k}_Wf'], W[f'c{k}_Ws'], W[f'c{k}_bf'], W[f'c{k}_bs']
        m = np.concatenate([Wf[2 * C:], Ws[2 * C:]], axis=1)
        return np.concatenate([m, np.concatenate([bf, bs])[None, :]], axis=0)

    onehotT = np.zeros((NCLS, G), np.float32)
    for g in range(G):
        onehotT[y[g], g] = 1.0

    L, BLOB = blob_layout(SL, NBLK, Bw)

    def bf16(a):
        return np.ascontiguousarray(a).astype(BF16NP)

    per_core = []
    for c in range(NCORE):
        parts = {
            'eaT': eaT[c].reshape(WPC * 33, Bw * 128).astype(FP8NP),
            'xT': np.ascontiguousarray(xT_full[:, c * NPC:(c + 1) * NPC]).astype(BF16NP),
            'clsOH': np.ascontiguousarray(clsOH[:, c * NPC:(c + 1) * NPC]).astype(BF16NP),
            'srcs': src_slab[c],
            'dsts': dst_slab[c],
            'dstloc': bf16(dstloc_a[c]),
            'deg': bf16(degp[c * NPC:(c + 1) * NPC].reshape(WPC, 128).T),
            'bt': bf16(gl[c * NPC:(c + 1) * NPC].reshape(WPC, 128).T),
            'invcnt': invcnt128,
            'W1': bf16(W['lin_W'][:C]),
            'B10': bf16(B10),
            # conv2 gate inputs scaled by 1/4 so the fp8e3 tables stay in
            # range for the larger h1; the act scales undo it on device
            'Wtab1': bf16(wtab(1)), 'Wtab2': bf16(wtab(2) * 0.25),
            'Wea1': bf16(wea(1)), 'Wea2': bf16(wea(2) * 0.25),
            'fc1a': np.ascontiguousarray(W['fc1_W'][:C]),
            'fc1b': np.ascontiguousarray(W['fc1_W'][C:]),
            'fc1bias': W['fc1_b'][None, :].copy(),
            'fc2aug': np.concatenate([W['fc2_W'], W['fc2_b'][None, :]], 0),
            'onehotT': onehotT,
            'ones64': np.ones((1, G), np.float32),
        }
        blob = np.zeros((1, BLOB), np.uint8)
        for name, (off, p, cc, esz) in L.items():
            if name == 'res':
                continue
            a = parts[name]
            assert a.shape == (p, cc) and a.dtype.itemsize == esz, \
                (name, a.shape, (p, cc), a.dtype)
            raw = np.frombuffer(np.ascontiguousarray(a).tobytes(), np.uint8)
            blob[0, off:off + raw.size] = raw
        per_core.append({'blob': blob})

    meta = dict(lowB=lowB, highB=highB, src_calls=src_calls,
                dst_calls=dst_calls, lowW=[int(v) for v in lowW],
                highW=[int(v) for v in highW], wbase16=wbase16,
                dbase16=dbase16)
    return per_core, meta


# ======================= bass kernel builder =======================

import concourse.bass as bass
import concourse.bacc as bacc
import concourse.tile as tile
from concourse import mybir, library_config

F32 = mybir.dt.float32
BF16 = mybir.dt.bfloat16
I16 = mybir.dt.int16
U8 = mybir.dt.uint8
F8 = mybir.dt.float8e3
F8E4 = mybir.dt.float8e4
AF = mybir.ActivationFunctionType

ABLATE = set()


def build(meta, stage="final"):
    lowB, highB = meta['lowB'], meta['highB']
    lowW, highW = meta['lowW'], meta['highW']
    wbase16, dbase16 = meta['wbase16'], meta['dbase16']
    Bw = lowB + highB
    SPW = Bw * 128
    NBLK = WPC * Bw
    SL = NBLK * 128
    src_calls = meta['src_calls']
    dst_calls = meta['dst_calls']
    L, BLOB = blob_layout(SL, NBLK, Bw)

    nc = bacc.Bacc("TRN2", target_bir_lowering=False, debug=False,
                   num_devices=NCORE, num_swdge_queues=4)
    blob_t = nc.dram_tensor("blob", [1, BLOB], U8, kind="ExternalInput")
    # single output: blob copy (loopback for device-residency across timed
    # calls) with the result written into the trailing 'res' region
    blob_o = nc.dram_tensor("blob_out", [1, BLOB], U8, kind="ExternalOutput")
    RES_OFF = L['res'][0]

    def view(name, dt):
        off, p, cc, esz = L[name]
        assert esz == mybir.dt.size(dt)
        return blob_t[0:1, off:off + p * cc * esz].bitcast(dt).rearrange(
            "o (p c) -> (o p) c", p=p)

    dbg = {}
    if stage == "h0":
        dbg['h0_own'] = nc.dram_tensor("dbg_h0", [NPC, C], BF16, kind="ExternalOutput")
    if stage in ("h1", "full"):
        dbg['h1_own'] = nc.dram_tensor("dbg_h1", [NPC, C], BF16, kind="ExternalOutput")
    if stage == "full":
        dbg['h2_own'] = nc.dram_tensor("dbg_h2", [NPC, C], BF16, kind="ExternalOutput")

    with tile.TileContext(nc) as tc:
        nc.gpsimd.load_library(library_config.mlp)
        ctx = contextlib.ExitStack()
        consts = ctx.enter_context(tc.tile_pool(name="consts", bufs=1))
        sbuf = ctx.enter_context(tc.tile_pool(name="sbuf", bufs=2))
        gates = ctx.enter_context(tc.tile_pool(name="gates", bufs=2))
        scratch = ctx.enter_context(tc.tile_pool(name="scratch", bufs=1))
        dram = ctx.enter_context(tc.tile_pool(name="dram", bufs=1, space="DRAM"))

        def load_const(name, shape, dt):
            t = consts.tile(shape, dt, tag=name + "_c")
            nc.sync.dma_start(t[:], view(name, dt))
            return t

        W1 = load_const('W1', [C, C], BF16)
        B10sb = load_const('B10', [11, C], BF16)
        Wtab = [load_const('Wtab1', [C, 512], BF16),
                load_const('Wtab2', [C, 512], BF16)]
        Wea = [load_const('Wea1', [33, 256], BF16),
               load_const('Wea2', [33, 256], BF16)]
        dstlocC = load_const('dstloc', [128, NBLK], BF16)
        degC = load_const('deg', [128, WPC], BF16)
        btC = load_const('bt', [128, WPC], BF16)
        invcntC = load_const('invcnt', [128, G], F32)
        fc1a = load_const('fc1a', [C, 32], F32)
        fc1b = load_const('fc1b', [NCLS, 32], F32)
        fc1bias = load_const('fc1bias', [1, 32], F32)
        fc2aug = load_const('fc2aug', [33, 1], F32)
        onehotT = load_const('onehotT', [NCLS, G], F32)
        ones64 = load_const('ones64', [1, G], F32)

        # idx slabs: shipped [16, SL/16], replicated to [128, SL/16] on device
        srcsl = consts.tile([128, SL // 16], I16, tag="srcsl")
        dstsl = consts.tile([128, SL // 16], I16, tag="dstsl")
        vs, vd = view('srcs', I16), view('dsts', I16)
        for kk in range(8):
            nc.sync.dma_start(srcsl[16 * kk:16 * kk + 16, :], vs)
            nc.sync.dma_start(dstsl[16 * kk:16 * kk + 16, :], vd)

        xTview = view('xT', BF16)
        clsOHview = view('clsOH', BF16)
        eaview = view('eaT', F8)

        # on-device iotas / identity
        it16 = consts.tile([128, Bw * 128], I16, tag="it16")
        nc.gpsimd.iota(it16[:], pattern=[[0, Bw], [1, 128]], channel_multiplier=0)
        iotaBw = consts.tile([128, Bw * 128], BF16, tag="iotaBw")
        nc.vector.tensor_copy(iotaBw[:], it16[:])
        ig16 = consts.tile([128, G], I16, tag="ig16")
        nc.gpsimd.iota(ig16[:], pattern=[[1, G]], channel_multiplier=0)
        iotaG = consts.tile([128, G], BF16, tag="iotaG")
        nc.vector.tensor_copy(iotaG[:], ig16[:])
        id16 = consts.tile([128, 128], I16, tag="id16")
        nc.gpsimd.iota(id16[:], pattern=[[1, 128]], channel_multiplier=-1)
        ident = consts.tile([128, 128], BF16, tag="ident")
        nc.vector.tensor_scalar(ident[:], id16[:], 0, None,
                                mybir.AluOpType.is_equal)
        identF8 = consts.tile([128, 128], F8, tag="identF8")
        nc.vector.tensor_scalar(identF8[:], id16[:], 0, None,
                                mybir.AluOpType.is_equal)

        # binary pooling one-hot [128, WPC, G]
        Sb01 = consts.tile([128, WPC, G], BF16, tag="Sb01")
        nc.vector.tensor_tensor(
            out=Sb01[:],
            in0=btC[:].rearrange("p (w o) -> p w o", o=1).to_broadcast([128, WPC, G]),
            in1=iotaG[:].rearrange("p (o g) -> p o g", o=1).to_broadcast([128, WPC, G]),
            op=mybir.AluOpType.is_equal)

        degC2 = consts.tile([128, WPC], BF16, tag="degC2")
        nc.vector.tensor_scalar(degC2[:], degC[:], 0.34609, None,
                                mybir.AluOpType.mult)
        h0own = consts.tile([128, WPC * 128], BF16, tag="h0own")
        h1own = consts.tile([128, WPC * 128], BF16, tag="h1own")

        # DRAM tiles (fp8 tables: halves gather traffic and AllGather payload)
        ts_sh = [dram.tile([NPC, 256], F8, name=f"ts_sh{k}", tag=f"ts_sh{k}")
                 for k in range(2)]
        td = [dram.tile([NPC, 256], F8, name=f"td{k}", tag=f"td{k}")
              for k in range(2)]
        ts_full = [dram.tile([NPAD, 256], F8, addr_space="Shared",
                             name=f"ts_full{k}", tag=f"ts_full{k}")
                   for k in range(2)]
        ar_in = dram.tile([128, G], F32)
        ar_out = dram.tile([128, G], F32, addr_space="Shared")
        sgt_dram = dram.tile([128, WPC * Bw * 128], F8E4, name="sgt_dram",
                             tag="sgt_dram")

        qn = [0]
        def next_q():
            q = qn[0] % 4
            qn[0] += 1
            return q

        # ================= PHASE A (own shard only) =================
        # loads/stores batched over 4-window groups: HWDGE charges ~630ns
        # fixed per dma_start, so fewer, larger DMAs shorten the startup
        PB = 4
        with tc.tile_pool(name="psA", bufs=2, space="PSUM") as psA:
            for w0 in range(0, WPC, PB):
                nb = min(PB, WPC - w0)
                xtb = sbuf.tile([128, PB * 128], BF16, tag="pAx")
                nc.sync.dma_start(xtb[:, 0:nb * 128],
                                  xTview[:, w0 * 128:(w0 + nb) * 128])
                ohb = sbuf.tile([11, PB * 128], BF16, tag="pAoh")
                nc.sync.dma_start(ohb[:, 0:nb * 128],
                                  clsOHview[:, w0 * 128:(w0 + nb) * 128])
                evb = sbuf.tile([128, PB, 512], F8, tag="pAev")
                for b in range(nb):
                    w = w0 + b
                    ps = psA.tile([128, 128], F32, tag="pA")
                    nc.tensor.matmul(out=ps[:], lhsT=W1[:],
                                     rhs=xtb[:, b * 128:(b + 1) * 128],
                                     start=True, stop=False)
                    nc.tensor.matmul(out=ps[:], lhsT=B10sb[:],
                                     rhs=ohb[:, b * 128:(b + 1) * 128],
                                     start=False, stop=True)
                    ho = sbuf.tile([128, 128], BF16, tag="pAout")
                    nc.scalar.activation(ho[:], ps[:], AF.Prelu, alpha=0.01)
                    ps2 = psA.tile([128, 512], F32, tag="pAtab")
                    nc.tensor.matmul(out=ps2[:], lhsT=ho[:], rhs=Wtab[0][:],
                                     start=True, stop=True)
                    nc.scalar.activation(evb[:, b, :], ps2[:], AF.Copy)
                    pst = psA.tile([128, 128], BF16, tag="pAtr")
                    nc.tensor.transpose(pst[:], ho[:], ident[:])
                    nc.vector.tensor_copy(h0own[:, w * 128:(w + 1) * 128],
                                          pst[:])
                nc.sync.dma_start(
                    td[0][w0 * 128:(w0 + nb) * 128, :].rearrange(
                        "(b p) e -> p b e", p=128),
                    evb[:, 0:nb, 0:256])
                nc.sync.dma_start(
                    ts_sh[0][w0 * 128:(w0 + nb) * 128, :].rearrange(
                        "(b p) e -> p b e", p=128),
                    evb[:, 0:nb, 256:512])

        if stage == "h0":
            for w in range(WPC):
                nc.sync.dma_start(dbg['h0_own'][w * 128:(w + 1) * 128, :],
                                  h0own[:, w * 128:(w + 1) * 128])
            ctx.close()
            return nc, dbg

        def allgather(k):
            if 'nocoll' in ABLATE:
                nc.sync.dma_start(ts_full[k][0:NPC, :], ts_sh[k][:])
            else:
                nc.gpsimd.collective_compute(
                    "AllGather", mybir.AluOpType.bypass,
                    replica_groups=[list(range(NCORE))],
                    ins=[ts_sh[k].opt()], outs=[ts_full[k].opt()])

        allgather(0)

        # scatter one-hot build for all windows (overlaps the AllGather);
        # both convs reload the stash instead of rebuilding
        for w in range(WPC):
            sgb = gates.tile([128, Bw, 128], BF16, tag="sgw_build")
            dlw = dstlocC[:, w * Bw:(w + 1) * Bw]
            nc.vector.tensor_tensor(
                out=sgb[:],
                in0=dlw[:].rearrange("p (b o) -> p b o", o=1).to_broadcast(
                    [128, Bw, 128]),
                in1=iotaBw[:].rearrange("p (b e) -> p b e", b=Bw),
                op=mybir.AluOpType.is_equal)
            sgf8 = gates.tile([128, Bw * 128], F8E4, tag="sgw_build8")
            nc.scalar.activation(sgf8[:], sgb[:].rearrange("p b e -> p (b e)"),
                                 AF.Copy)
            nc.sync.dma_start(sgt_dram[:, w * Bw * 128:(w + 1) * Bw * 128],
                              sgf8[:])

        # blob loopback copy: issued here so the transfer overlaps conv1
        nc.sync.dma_start(blob_o[0:1, 0:RES_OFF], blob_t[0:1, 0:RES_OFF])

        # ================= CONV =================
        def conv(k, hprev_own, hout_own, leaky, psGate, psAgg, psFlush, pool_mm):
            tsF, tdF = ts_full[k], td[k]
            tsc = 4.0 if k == 1 else 1.0   # undo the 1/4 host scaling of conv2
            for w in range(WPC):
                bw = int(lowW[w]) + int(highW[w])
                tsg = sbuf.tile([128, Bw, 256], F8, tag="tsg")
                tdg = sbuf.tile([128, Bw, 256], F8, tag="tdg")
                base16 = wbase16[w]
                dbs16 = dbase16[w]
                if 'gather' not in ABLATE:
                    for (aoff, n, tbl) in src_calls[w]:
                        s0 = (aoff - base16) * 16
                        in_ap = tsF[0:S0, :] if tbl == 0 else tsF[S0:NPAD, :]
                        nc.gpsimd.dma_gather(
                            out_ap=tsg[:, s0 // 128: s0 // 128 + n // 128, :],
                            in_ap=in_ap,
                            idxs_ap=srcsl[:, aoff:aoff + n // 16],
                            num_idxs=n, num_idxs_reg=n, elem_size=256,
                            queue_num=next_q())
                    for (aoff, n) in dst_calls[w]:
                        s0 = (aoff - dbs16) * 16
                        nc.gpsimd.dma_gather(
                            out_ap=tdg[:, s0 // 128: s0 // 128 + n // 128, :],
                            in_ap=tdF[:],
                            idxs_ap=dstsl[:, aoff:aoff + n // 16],
                            num_idxs=n, num_idxs_reg=n, elem_size=256,
                            queue_num=next_q())
                eaw = sbuf.tile([33, Bw * 128], F8, tag="eaw")
                nc.sync.dma_start(eaw[:, 0:bw * 128],
                                  eaview[w * 33:(w + 1) * 33, 0:bw * 128])

                agg = psAgg.tile([128, 256], F32, tag="agg")
                gbw = gates.tile([128, Bw, 256], BF16, tag="gbw")
                for g0 in range(0, bw, 4):
                    ng = min(4, bw - g0)
                    ps = psGate.tile([128, 1024], F32, tag="gate")
                    # src tables injected into PSUM by the tensor engine;
                    # keeps the big DVE add off the critical path
                    for b in range(ng):
                        blk = g0 + b
                        nc.tensor.matmul(
                            out=ps[:, b * 256:(b + 1) * 256],
                            lhsT=eaw[:, blk * 128:(blk + 1) * 128],
                            rhs=Wea[k][:], start=True, stop=False)
                    for b in range(ng):
                        blk = g0 + b
                        nc.tensor.matmul(
                            out=ps[:, b * 256:(b + 1) * 256],
                            lhsT=identF8[:], rhs=tsg[:, blk, :],
                            start=False, stop=True)
                    nc.vector.tensor_add(
                        gbw[:, g0:g0 + ng, :].rearrange("p b e -> p (b e)"),
                        ps[:, :ng * 256],
                        tdg[:, g0:g0 + ng, :].rearrange("p b e -> p (b e)"))
                if w % 2 == 0:
                    sgw4 = gates.tile([128, 2 * SPW], F8E4, tag="sgw4")
                    whi = min(WPC, w + 2)
                    nc.sync.dma_start(sgw4[:, 0:(whi - w) * SPW],
                                      sgt_dram[:, w * SPW:whi * SPW])
                sgb0 = (w % 2) * SPW
                wtaw = gates.tile([128, Bw, 256], BF16, tag="wtaw")
                nc.scalar.activation(wtaw[:, 0:bw, 128:256],
                                     gbw[:, 0:bw, 0:128],
                                     AF.Tanh, scale=0.5 * tsc)
                slw = scratch.tile([128, Bw, 128], BF16, tag="slw")
                nc.scalar.activation(slw[:, 0:bw, :], gbw[:, 0:bw, 128:256],
                                     AF.Silu, scale=tsc)
                tbw = scratch.tile([128, Bw, 128], BF16, tag="tbw")
                nc.scalar.activation(tbw[:, 0:bw, :], gbw[:, 0:bw, 128:256],
                                     AF.Tanh, scale=0.42077 * tsc)
                sqw = scratch.tile([128, Bw, 128], BF16, tag="sqw")
                nc.scalar.activation(sqw[:, 0:bw, :], tbw[:, 0:bw, :],
                                     AF.Square, scale=0.83197)
                # TT/TS ops (2x/4x DVE modes) instead of STT (always 1x)
                vw = scratch.tile([128, Bw, 128], BF16, tag="vw")
                nc.vector.tensor_tensor(out=vw[:, 0:bw, :],
                                        in0=slw[:, 0:bw, :],
                                        in1=sqw[:, 0:bw, :],
                                        op=mybir.AluOpType.subtract)
                tp1 = scratch.tile([128, Bw, 128], BF16, tag="tp1")
                nc.vector.tensor_scalar(tp1[:, 0:bw, :],
                                        wtaw[:, 0:bw, 128:256], 1.0, None,
                                        mybir.AluOpType.add)
                nc.vector.tensor_tensor(out=wtaw[:, 0:bw, 0:128],
                                        in0=tp1[:, 0:bw, :],
                                        in1=vw[:, 0:bw, :],
                                        op=mybir.AluOpType.mult)
                for blk in range(bw):
                    nc.tensor.matmul(
                        out=agg[:],
                        lhsT=sgw4[:, sgb0 + blk * 128:sgb0 + (blk + 1) * 128],
                        rhs=wtaw[:, blk, :],
                        start=(blk == 0), stop=(blk == bw - 1))
                t2 = sbuf.tile([128, 128], F32, tag="t2")
                nc.vector.tensor_tensor(
                    out=t2[:], in0=degC2[:, w:w + 1].to_broadcast([128, 128]),
                    in1=hprev_own[:, w * 128:(w + 1) * 128],
                    op=mybir.AluOpType.add)
                t3 = sbuf.tile([128, 128], F32, tag="t3")
                nc.vector.scalar_tensor_tensor(
                    out=t3[:], in0=agg[:, 128:256], scalar=0.34609,
                    in1=t2[:], op0=mybir.AluOpType.mult,
                    op1=mybir.AluOpType.add)
                hsum = sbuf.tile([128, 128], F32, tag="hsum")
                nc.vector.scalar_tensor_tensor(
                    out=hsum[:], in0=agg[:, 0:128], scalar=0.5,
                    in1=t3[:], op0=mybir.AluOpType.mult,
                    op1=mybir.AluOpType.add)
                hw = hout_own[:, w * 128:(w + 1) * 128]
                if leaky:
                    nc.scalar.activation(hw, hsum[:], AF.Prelu, alpha=0.01)
                else:
                    nc.scalar.activation(hw, hsum[:], AF.Copy)
                if k == 0:
                    pst = psFlush.tile([128, 128], BF16, tag="flushtr")
                    nc.tensor.transpose(pst[:], hw, ident[:])
                    h1T = sbuf.tile([128, 128], BF16, tag="h1T")
                    nc.scalar.activation(h1T[:], pst[:], AF.Copy)
                    ps2 = psFlush.tile([128, 512], F32, tag="flushtab")
                    nc.tensor.matmul(out=ps2[:], lhsT=h1T[:], rhs=Wtab[1][:],
                                     start=True, stop=True)
                    if w % 2 == 0:
                        evb2 = sbuf.tile([128, 2, 512], F8, tag="flushev")
                    nc.scalar.activation(evb2[:, w % 2, :], ps2[:], AF.Copy)
                    if w % 2 == 1 or w == WPC - 1:
                        nb2 = w % 2 + 1
                        w0f = w - w % 2
                        nc.sync.dma_start(
                            td[1][w0f * 128:(w0f + nb2) * 128, :].rearrange(
                                "(b p) e -> p b e", p=128),
                            evb2[:, 0:nb2, 0:256])
                        nc.sync.dma_start(
                            ts_sh[1][w0f * 128:(w0f + nb2) * 128, :].rearrange(
                                "(b p) e -> p b e", p=128),
                            evb2[:, 0:nb2, 256:512])
                else:
                    nc.tensor.matmul(out=pool_mm[:, :G], lhsT=hw,
                                     rhs=Sb01[:, w, :],
                                     start=(w == 0), stop=(w == WPC - 1))

        with tc.tile_pool(name="psG1", bufs=2, space="PSUM") as psG1, \
             tc.tile_pool(name="psA1", bufs=2, space="PSUM") as psA1, \
             tc.tile_pool(name="psF1", bufs=1, space="PSUM") as psF1:
            conv(0, h0own, h1own, True, psG1, psA1, psF1, None)

        if stage == "h1":
            for w in range(WPC):
                nc.sync.dma_start(dbg['h1_own'][w * 128:(w + 1) * 128, :],
                                  h1own[:, w * 128:(w + 1) * 128])
            ctx.close()
            return nc, dbg

        allgather(1)

        h2own = h0own
        with tc.tile_pool(name="psPool", bufs=1, space="PSUM") as psPool:
            pool_mm = psPool.tile([128, G], F32, tag="pool")
            with tc.tile_pool(name="psG2", bufs=2, space="PSUM") as psG2, \
                 tc.tile_pool(name="psA2", bufs=2, space="PSUM") as psA2:
                conv(1, h1own, h2own, False, psG2, psA2, None, pool_mm)

            if stage == "full":
                for w in range(WPC):
                    nc.sync.dma_start(dbg['h1_own'][w * 128:(w + 1) * 128, :],
                                      h1own[:, w * 128:(w + 1) * 128])
                    nc.sync.dma_start(dbg['h2_own'][w * 128:(w + 1) * 128, :],
                                      h2own[:, w * 128:(w + 1) * 128])

            poolsb = sbuf.tile([128, G], F32, tag="poolsb")
            nc.vector.tensor_copy(poolsb[:], pool_mm[:])
        nc.sync.dma_start(ar_in[:], poolsb[:])
        if 'nocoll' in ABLATE:
            nc.sync.dma_start(ar_out[:], ar_in[:])
        else:
            nc.gpsimd.collective_compute(
                "AllReduce", mybir.AluOpType.add,
                replica_groups=[list(range(NCORE))],
                ins=[ar_in.opt()], outs=[ar_out.opt()])

        with tc.tile_pool(name="psH", bufs=1, space="PSUM") as psH:
            pooled = sbuf.tile([128, G], F32, tag="pooled")
            nc.sync.dma_start(pooled[:], ar_out[:])
            nc.vector.tensor_tensor(out=pooled[:], in0=pooled[:],
                                    in1=invcntC[:], op=mybir.AluOpType.mult)
            hps = psH.tile([32, G], F32, tag="head1")
            nc.tensor.matmul(out=hps[:], lhsT=fc1a[:], rhs=pooled[:],
                             start=True, stop=False)
            nc.tensor.matmul(out=hps[:], lhsT=fc1b[:], rhs=onehotT[:],
                             start=False, stop=False)
            nc.tensor.matmul(out=hps[:], lhsT=fc1bias[:], rhs=ones64[:],
                             start=False, stop=True)
            a1 = sbuf.tile([33, G], F32, tag="a1")
            nc.scalar.activation(a1[0:32, :], hps[:], AF.Prelu, alpha=0.01)
            nc.vector.memset(a1[32:33, :], 1.0)
            hps2 = psH.tile([1, G], F32, tag="head2")
            nc.tensor.matmul(out=hps2[:], lhsT=fc2aug[:], rhs=a1[:],
                             start=True, stop=True)
            rest = sbuf.tile([1, G], F32, tag="rest")
            nc.scalar.activation(rest[:], hps2[:], AF.Tanh, scale=0.5)
            res = sbuf.tile([1, G], F32, tag="res")
            nc.vector.tensor_scalar(res[:], rest[:], 0.5, 0.5,
                                    mybir.AluOpType.mult,
                                    mybir.AluOpType.add)
            nc.sync.dma_start(
                blob_o[0:1, RES_OFF:RES_OFF + G * 4].bitcast(F32), res[:])

        ctx.close()
    return nc, dbg


# ======================= entry point =======================
_CACHE = {}


def _get_compiled(meta_key, meta):
    if meta_key not in _CACHE:
        nc, _ = build(meta, stage="final")
        nc.compile()
        _CACHE[meta_key] = nc
    return _CACHE[meta_key]


def make_inputs(inputs):
    return prep(inputs)


def kernel(**inputs) -> np.ndarray:
    per_core, meta = make_inputs(inputs)
    key = (meta['lowB'], meta['highB'])
    nc = _get_compiled(key, meta)
    from concourse.bass_utils import run_bass_kernel_spmd
    res = run_bass_kernel_spmd(nc, per_core, core_ids=list(range(NCORE)))
    Bw = meta['lowB'] + meta['highB']
    L, _ = blob_layout(WPC * Bw * 128, WPC * Bw, Bw)
    ro = L['res'][0]
    raw = np.asarray(res.results[0]['blob_out']).reshape(-1)[ro:ro + G * 4]
    return raw.view(np.float32).reshape(G, 1).astype(np.float32)

